# revision 11
# baseline (speedup 1.0000x reference)
"""Causal self-attention (B=4, T=2048, D=1024, H=16) on 8 TRN2 NeuronCores.

Sharding: core c handles batch b = c//2 and head-group g = c%2 (8 heads each).
Each core computes, for its (b, g):
    qkv_loc = x[b] @ w_qkv[:, cols(g)]          (q|k|v local, 512 cols each)
    att     = causal_attention(q, k, v)          (8 heads, hd=64)
    y_part  = att @ w_out[rows(g), :]            ([2048, 1024] partial)
Host sums the two partial outputs per batch.

All compute is done in MM_DTYPE on the TensorEngine (PSUM accumulation is
fp32); softmax uses exp on ScalarE with deferred normalization (rowsums via
an appended ones-column in V, reciprocal broadcast via a K=1 outer-product
matmul).
"""

import math
import os

import numpy as np

import concourse.bass as bass
import concourse.mybir as mybir
from concourse import bacc, tile
from concourse import bass_utils
from concourse.masks import make_identity

# Problem constants (hardcoded per contest contract)
B = 4
T = 2048
D = 1024
H = 16
HD = 64
H_LOC = 8               # heads per core
CLOC = H_LOC * HD       # 512 local head dims
P = 128
N_CORES = 8

F32 = mybir.dt.float32

# Compute dtype knob: "bf16" | "f32" | "f32r"
MM_MODE = os.environ.get("ATTN_MM_MODE", "bf16")
_MM_MAP = {
    "bf16": mybir.dt.bfloat16,
    "f32": mybir.dt.float32,
    "f32r": mybir.dt.float32r,
}


def _build_kernel_body(nc, tc, x_ap, wqkv_ap, wout_ap, out_ap, mm):
    from contextlib import ExitStack

    Exp = mybir.ActivationFunctionType.Exp
    mult = mybir.AluOpType.mult

    needs_cast = mm != F32

    ctx = ExitStack()
    # ---------------- persistent tiles ----------------
    const = ctx.enter_context(tc.tile_pool(name="const", bufs=1))
    ident = const.tile([P, P], mm)
    make_identity(nc, ident)

    # causal mask helper: wm[p, x] = 1.0 iff p <= x - 384 else 0.0
    wm = const.tile([P, 896], mm)
    nc.gpsimd.memset(wm, 1.0)
    # keep 1.0 where (f - p - 384) >= 0, i.e. p <= f - 384; else fill 0
    nc.gpsimd.affine_select(
        out=wm,
        in_=wm,
        compare_op=mybir.AluOpType.is_ge,
        fill=0.0,
        base=-384,
        channel_multiplier=-1,
        pattern=[[1, 896]],
    )

    oc = const.tile([1, 64], mm)  # ones column for rowsum broadcast
    nc.gpsimd.memset(oc, 1.0)

    qkt_pool = ctx.enter_context(tc.tile_pool(name="qkt", bufs=1))
    QT = qkt_pool.tile([P, 4, T], mm)   # head h -> rows (h%2)*64.., subtile h//2
    KT = qkt_pool.tile([P, 4, T], mm)
    V_aug = qkt_pool.tile([P, 16, H_LOC, HD + 1], mm)  # [j%128, jb, h, dd|ones]
    nc.gpsimd.memset(V_aug[:, :, :, HD], 1.0)


    # ---------------- phase A: x -> xT (transposed, cast) ----------------
    xa = x_ap.rearrange("(tb p) d -> tb p d", p=P)  # [16, 128, 1024]

    wqk = wqkv_ap[:, 0:2 * CLOC].rearrange("(o p) c -> p o c", p=P)
    wv = wqkv_ap[:, 2 * CLOC:3 * CLOC].rearrange("(o p) c -> p o c", p=P)

    with tc.tile_pool(name="xt", bufs=1) as xt_pool, \
         tc.tile_pool(name="psAB", bufs=2, space="PSUM") as psum:
        xT = xt_pool.tile([P, 8, T], mm)  # [d%128, d//128, t]

        # ---------------- phase A: x -> xT (transposed, cast) ----------------
        with tc.tile_pool(name="lda", bufs=2) as lda:
            for tb in range(T // P):
                xin = lda.tile([P, D], F32, tag="xin")
                nc.sync.dma_start(xin, xa[tb])
                if needs_cast:
                    xc = lda.tile([P, D], mm, tag="xc")
                    nc.vector.tensor_copy(xc, xin)
                else:
                    xc = xin
                for db in range(D // P):
                    pt = psum.tile([P, P], F32, tag="ps_t")
                    nc.tensor.transpose(pt, xc[:, db * P:(db + 1) * P], ident)
                    nc.vector.tensor_copy(xT[:, db, tb * P:(tb + 1) * P], pt)

        # ---------------- phase B: QKV projection ----------------
        # Q^T / K^T: psum[c_block 128, t 512] = w_cb.T @ xT
        with tc.tile_pool(name="ldw", bufs=2) as ldw:
            for cb in range(8):
                wst = ldw.tile([P, 8, P], F32, tag="wst")
                nc.sync.dma_start(wst, wqk[:, :, cb * P:(cb + 1) * P])
                if needs_cast:
                    wcb = ldw.tile([P, 8, P], mm, tag="wcb")
                    nc.vector.tensor_copy(wcb, wst)
                else:
                    wcb = wst
                dest = QT if cb < 4 else KT
                sub = cb % 4
                for it in range(4):
                    ps = psum.tile([P, 512], F32, tag="ps_qkv")
                    for k in range(8):
                        nc.tensor.matmul(
                            ps,
                            wcb[:, k, :],
                            xT[:, k, it * 512:(it + 1) * 512],
                            start=(k == 0),
                            stop=(k == 7),
                        )
                    nc.vector.tensor_copy(dest[:, sub, it * 512:(it + 1) * 512], ps)

        # V projection: psum[t 128, c 256] = x @ w_v (two c-halves)
        for half in range(2):
            ch = CLOC // 2
            with tc.tile_pool(name="ldv", bufs=1) as ldv:
                wv_st = ldv.tile([P, 8, ch], F32, tag="wv_st")
                nc.sync.dma_start(wv_st, wv[:, :, half * ch:(half + 1) * ch])
                if needs_cast:
                    wv_sb = ldv.tile([P, 8, ch], mm, tag="wv_sb")
                    nc.vector.tensor_copy(wv_sb, wv_st)
                else:
                    wv_sb = wv_st
                for tb in range(T // P):
                    ps = psum.tile([P, ch], F32, tag="ps_v")
                    for k in range(8):
                        nc.tensor.matmul(
                            ps,
                            xT[:, k, tb * P:(tb + 1) * P],
                            wv_sb[:, k, :],
                            start=(k == 0),
                            stop=(k == 7),
                        )
                    nc.vector.tensor_copy(
                        V_aug[:, tb, half * 4:(half + 1) * 4, 0:HD],
                        ps.rearrange("p (h d) -> p h d", h=H_LOC // 2),
                    )

    # ---------------- phase C: causal attention ----------------
    AT = qkt_pool.tile([P, 4, T], mm)   # attention output, transposed like QT
    with tc.tile_pool(name="att", bufs=3) as att_pool, \
         tc.tile_pool(name="attsm", bufs=2) as sm_pool, \
         tc.tile_pool(name="psC", bufs=2, space="PSUM") as psum:
        for h in range(H_LOC):
            row0 = (h % 2) * 64
            sub = h // 2
            QTh = QT[row0:row0 + 64, sub, :]
            KTh = KT[row0:row0 + 64, sub, :]
            for it in range(4):
                i0 = it * 512
                njb = 4 * (it + 1)
                po = psum.tile([P, 512], F32, tag="ps_o")
                for jb in range(njb):
                    ps = psum.tile([P, 512], F32, tag="ps_s")
                    nc.tensor.matmul(
                        ps,
                        KTh[:, jb * P:(jb + 1) * P],
                        QTh[:, i0:i0 + 512],
                        start=True,
                        stop=True,
                    )
                    es = att_pool.tile([P, 512], mm, tag="es")
                    nc.scalar.activation(es, ps, Exp, scale=0.125)
                    off = jb * P - i0
                    if off >= 0:  # diagonal region: zero out j > i
                        s = 384 - off
                        nc.vector.tensor_tensor(es, es, wm[:, s:s + 512], mult)
                    nc.tensor.matmul(
                        po[0:HD + 1, :],
                        V_aug[:, jb, h, :],
                        es,
                        start=(jb == 0),
                        stop=(jb == njb - 1),
                    )
                # deferred softmax normalization
                rr = sm_pool.tile([1, 512], F32, tag="rr")
                nc.vector.tensor_copy(rr, po[HD:HD + 1, :])
                nc.vector.reciprocal(rr, rr)
                if needs_cast:
                    rm = sm_pool.tile([1, 512], mm, tag="rm")
                    nc.vector.tensor_copy(rm, rr)
                else:
                    rm = rr
                pb = psum.tile([64, 512], F32, tag="ps_b")
                nc.tensor.matmul(pb, oc, rm, start=True, stop=True)
                rb = sm_pool.tile([64, 512], F32, tag="rb")
                nc.vector.tensor_copy(rb, pb)
                nc.vector.tensor_tensor(
                    AT[row0:row0 + 64, sub, i0:i0 + 512],
                    po[0:64, :],
                    rb,
                    mult,
                )

    # ---------------- phase D: output projection ----------------
    wo = wout_ap.rearrange("(o p) n -> p o n", p=P)  # [128, 4, 1024]
    oa = out_ap.rearrange("(tb p) d -> tb p d", p=P)
    with tc.tile_pool(name="ldo", bufs=2) as ldo, \
         tc.tile_pool(name="ypool", bufs=3) as ypool, \
         tc.tile_pool(name="psD", bufs=4, space="PSUM") as psum:
        wo_st = ldo.tile([P, 4, D], F32, tag="wo_st")
        nc.sync.dma_start(wo_st, wo)
        if needs_cast:
            wo_sb = ldo.tile([P, 4, D], mm, tag="wo_sb")
            nc.vector.tensor_copy(wo_sb, wo_st)
        else:
            wo_sb = wo_st
        for tb in range(T // P):
            for nt in range(2):
                py = psum.tile([P, 512], F32, tag="ps_y")
                for k in range(4):
                    nc.tensor.matmul(
                        py,
                        AT[:, k, tb * P:(tb + 1) * P],
                        wo_sb[:, k, nt * 512:(nt + 1) * 512],
                        start=(k == 0),
                        stop=(k == 3),
                    )
                ysb = ypool.tile([P, 512], F32, tag="ysb")
                nc.vector.tensor_copy(ysb, py)
                nc.sync.dma_start(oa[tb, :, nt * 512:(nt + 1) * 512], ysb)

    ctx.close()


_CACHE = {}


def _get_nc(mode=None):
    mode = mode or MM_MODE
    if mode in _CACHE:
        return _CACHE[mode]
    mm = _MM_MAP[mode]
    nc = bacc.Bacc(
        "TRN2",
        target_bir_lowering=False,
        debug=False,
        enable_asserts=False,
        num_devices=N_CORES,
    )
    x_d = nc.dram_tensor("x", [T, D], F32, kind="ExternalInput")
    wqkv_d = nc.dram_tensor("w_qkv", [D, 3 * CLOC], F32, kind="ExternalInput")
    wout_d = nc.dram_tensor("w_out", [CLOC, D], F32, kind="ExternalInput")
    out_d = nc.dram_tensor("out", [T, D], F32, kind="ExternalOutput")
    with tile.TileContext(nc) as tc:
        _build_kernel_body(
            nc, tc, x_d.ap(), wqkv_d.ap(), wout_d.ap(), out_d.ap(), mm
        )
    nc.compile()
    _CACHE[mode] = nc
    return nc


def _make_in_maps(x, w_qkv, w_out):
    x = np.ascontiguousarray(np.asarray(x, dtype=np.float32))
    w_qkv = np.ascontiguousarray(np.asarray(w_qkv, dtype=np.float32))
    w_out = np.ascontiguousarray(np.asarray(w_out, dtype=np.float32))
    in_maps = []
    for c in range(N_CORES):
        b, g = divmod(c, 2)
        c0 = g * CLOC
        wloc = np.concatenate(
            [
                w_qkv[:, c0:c0 + CLOC],
                w_qkv[:, D + c0:D + c0 + CLOC],
                w_qkv[:, 2 * D + c0:2 * D + c0 + CLOC],
            ],
            axis=1,
        )
        in_maps.append({
            "x": np.ascontiguousarray(x[b]),
            "w_qkv": np.ascontiguousarray(wloc),
            "w_out": np.ascontiguousarray(w_out[c0:c0 + CLOC]),
        })
    return in_maps


def run(x, w_qkv, w_out, trace=False, mode=None):
    nc = _get_nc(mode)
    in_maps = _make_in_maps(x, w_qkv, w_out)
    res = bass_utils.run_bass_kernel_spmd(
        nc, in_maps, core_ids=list(range(N_CORES)), trace=trace
    )
    y = np.empty((B, T, D), dtype=np.float32)
    for b in range(B):
        y[b] = res.results[2 * b]["out"] + res.results[2 * b + 1]["out"]
    return y, res


def kernel(x, w_qkv, w_out):
    y, _ = run(x, w_qkv, w_out, trace=False)
    return y


# revision 12
# speedup vs baseline: 2.1953x; 2.1953x over previous
"""Causal self-attention (B=4, T=2048, D=1024, H=16) on 8 TRN2 NeuronCores.

Sharding: core c handles batch b = c//2 and head-group g = c%2 (8 heads each).
Each core computes, for its (b, g):
    qkv_loc = x[b] @ w_qkv[:, cols(g)]          (q|k|v local, 512 cols each)
    att     = causal_attention(q, k, v)          (8 heads, hd=64)
    y_part  = att @ w_out[rows(g), :]            ([2048, 1024] partial)
Host sums the two partial outputs per batch.

All compute is done in MM_DTYPE on the TensorEngine (PSUM accumulation is
fp32); softmax uses exp on ScalarE with deferred normalization (rowsums via
an appended ones-column in V, reciprocal broadcast via a K=1 outer-product
matmul).
"""

import math
import os

import numpy as np

import concourse.bass as bass
import concourse.mybir as mybir
from concourse import bacc, tile
from concourse import bass_utils
from concourse.masks import make_identity

# Problem constants (hardcoded per contest contract)
B = 4
T = 2048
D = 1024
H = 16
HD = 64
H_LOC = 8               # heads per core
CLOC = H_LOC * HD       # 512 local head dims
P = 128
N_CORES = 8

F32 = mybir.dt.float32

# Compute dtype knob: "bf16" | "f32" | "f32r"
MM_MODE = os.environ.get("ATTN_MM_MODE", "bf16")
_MM_MAP = {
    "bf16": mybir.dt.bfloat16,
    "f32": mybir.dt.float32,
    "f32r": mybir.dt.float32r,
}


def _build_kernel_body(nc, tc, x_ap, wqkv_ap, wout_ap, out_ap, mm):
    from contextlib import ExitStack

    Exp = mybir.ActivationFunctionType.Exp
    mult = mybir.AluOpType.mult

    needs_cast = mm != F32

    ctx = ExitStack()
    # ---------------- persistent tiles ----------------
    const = ctx.enter_context(tc.tile_pool(name="const", bufs=1))
    ident = const.tile([P, P], mm)
    make_identity(nc, ident)

    # causal mask helper: wm[p, x] = 1.0 iff p <= x - 384 else 0.0
    wm = const.tile([P, 896], mm)
    nc.gpsimd.memset(wm, 1.0)
    # keep 1.0 where (f - p - 384) >= 0, i.e. p <= f - 384; else fill 0
    nc.gpsimd.affine_select(
        out=wm,
        in_=wm,
        compare_op=mybir.AluOpType.is_ge,
        fill=0.0,
        base=-384,
        channel_multiplier=-1,
        pattern=[[1, 896]],
    )

    oc = const.tile([1, 64], mm)  # ones column for rowsum broadcast
    nc.gpsimd.memset(oc, 1.0)

    qkt_pool = ctx.enter_context(tc.tile_pool(name="qkt", bufs=1))
    QT = qkt_pool.tile([P, 4, T], mm)   # head h -> rows (h%2)*64.., subtile h//2
    KT = qkt_pool.tile([P, 4, T], mm)
    V_aug = qkt_pool.tile([P, 16, H_LOC, HD + 1], mm)  # [j%128, jb, h, dd|ones]
    nc.gpsimd.memset(V_aug[:, :, :, HD], 1.0)


    # ---------------- phase A: x -> xT (transposed, cast) ----------------
    xa = x_ap.rearrange("(tb p) d -> tb p d", p=P)  # [16, 128, 1024]

    wqk = wqkv_ap[:, 0:2 * CLOC].rearrange("(o p) c -> p o c", p=P)
    wv = wqkv_ap[:, 2 * CLOC:3 * CLOC].rearrange("(o p) c -> p o c", p=P)

    with tc.tile_pool(name="xt", bufs=1) as xt_pool, \
         tc.tile_pool(name="psAB", bufs=2, space="PSUM") as psum:
        xT = xt_pool.tile([P, 8, T], mm)  # [d%128, d//128, t]

        # ---------------- phase A: x -> xT (transposed, cast) ----------------
        with tc.tile_pool(name="lda", bufs=2) as lda:
            for tb in range(T // P):
                xin = lda.tile([P, D], F32, tag="xin")
                nc.sync.dma_start(xin, xa[tb])
                if needs_cast:
                    xc = lda.tile([P, D], mm, tag="xc")
                    nc.vector.tensor_copy(xc, xin)
                else:
                    xc = xin
                for db in range(D // P):
                    pt = psum.tile([P, P], mm, tag="ps_t")
                    nc.tensor.transpose(pt, xc[:, db * P:(db + 1) * P], ident)
                    nc.vector.tensor_copy(xT[:, db, tb * P:(tb + 1) * P], pt)

        # ---------------- phase B: QKV projection ----------------
        # Q^T / K^T: psum[c_block 128, t 512] = w_cb.T @ xT
        with tc.tile_pool(name="ldw", bufs=2) as ldw:
            for cb in range(8):
                wst = ldw.tile([P, 8, P], F32, tag="wst")
                nc.sync.dma_start(wst, wqk[:, :, cb * P:(cb + 1) * P])
                if needs_cast:
                    wcb = ldw.tile([P, 8, P], mm, tag="wcb")
                    nc.vector.tensor_copy(wcb, wst)
                else:
                    wcb = wst
                dest = QT if cb < 4 else KT
                sub = cb % 4
                for it in range(4):
                    ps = psum.tile([P, 512], F32, tag="ps_qkv")
                    for k in range(8):
                        nc.tensor.matmul(
                            ps,
                            wcb[:, k, :],
                            xT[:, k, it * 512:(it + 1) * 512],
                            start=(k == 0),
                            stop=(k == 7),
                        )
                    nc.vector.tensor_copy(dest[:, sub, it * 512:(it + 1) * 512], ps)

        # V projection: psum[t 128, c 256] = x @ w_v (two c-halves)
        for half in range(2):
            ch = CLOC // 2
            with tc.tile_pool(name="ldv", bufs=1) as ldv:
                wv_st = ldv.tile([P, 8, ch], F32, tag="wv_st")
                nc.sync.dma_start(wv_st, wv[:, :, half * ch:(half + 1) * ch])
                if needs_cast:
                    wv_sb = ldv.tile([P, 8, ch], mm, tag="wv_sb")
                    nc.vector.tensor_copy(wv_sb, wv_st)
                else:
                    wv_sb = wv_st
                for tb in range(T // P):
                    ps = psum.tile([P, ch], F32, tag="ps_v")
                    for k in range(8):
                        nc.tensor.matmul(
                            ps,
                            xT[:, k, tb * P:(tb + 1) * P],
                            wv_sb[:, k, :],
                            start=(k == 0),
                            stop=(k == 7),
                        )
                    nc.vector.tensor_copy(
                        V_aug[:, tb, half * 4:(half + 1) * 4, 0:HD],
                        ps.rearrange("p (h d) -> p h d", h=H_LOC // 2),
                    )

    # ---------------- phase C: causal attention ----------------
    AT = qkt_pool.tile([P, 4, T], mm)   # attention output, transposed like QT
    with tc.tile_pool(name="att", bufs=3) as att_pool, \
         tc.tile_pool(name="attsm", bufs=2) as sm_pool, \
         tc.tile_pool(name="psC", bufs=2, space="PSUM") as psum:
        for h in range(H_LOC):
            row0 = (h % 2) * 64
            sub = h // 2
            QTh = QT[row0:row0 + 64, sub, :]
            KTh = KT[row0:row0 + 64, sub, :]
            for it in range(4):
                i0 = it * 512
                njb = 4 * (it + 1)
                po = psum.tile([P, 512], F32, tag="ps_o")
                for jb in range(njb):
                    ps = psum.tile([P, 512], F32, tag="ps_s")
                    nc.tensor.matmul(
                        ps,
                        KTh[:, jb * P:(jb + 1) * P],
                        QTh[:, i0:i0 + 512],
                        start=True,
                        stop=True,
                    )
                    es = att_pool.tile([P, 512], mm, tag="es")
                    nc.scalar.activation(es, ps, Exp, scale=0.125)
                    off = jb * P - i0
                    if off >= 0:  # diagonal region: zero out j > i
                        s = 384 - off
                        nc.vector.tensor_tensor(es, es, wm[:, s:s + 512], mult)
                    nc.tensor.matmul(
                        po[0:HD + 1, :],
                        V_aug[:, jb, h, :],
                        es,
                        start=(jb == 0),
                        stop=(jb == njb - 1),
                    )
                # deferred softmax normalization
                rr = sm_pool.tile([1, 512], F32, tag="rr")
                nc.vector.tensor_copy(rr, po[HD:HD + 1, :])
                nc.vector.reciprocal(rr, rr)
                if needs_cast:
                    rm = sm_pool.tile([1, 512], mm, tag="rm")
                    nc.vector.tensor_copy(rm, rr)
                else:
                    rm = rr
                pb = psum.tile([64, 512], F32, tag="ps_b")
                nc.tensor.matmul(pb, oc, rm, start=True, stop=True)
                rb = sm_pool.tile([64, 512], F32, tag="rb")
                nc.vector.tensor_copy(rb, pb)
                nc.vector.tensor_tensor(
                    AT[row0:row0 + 64, sub, i0:i0 + 512],
                    po[0:64, :],
                    rb,
                    mult,
                )

    # ---------------- phase D: output projection ----------------
    wo = wout_ap.rearrange("(o p) n -> p o n", p=P)  # [128, 4, 1024]
    oa = out_ap.rearrange("(tb p) d -> tb p d", p=P)
    with tc.tile_pool(name="ldo", bufs=2) as ldo, \
         tc.tile_pool(name="ypool", bufs=3) as ypool, \
         tc.tile_pool(name="psD", bufs=4, space="PSUM") as psum:
        wo_st = ldo.tile([P, 4, D], F32, tag="wo_st")
        nc.sync.dma_start(wo_st, wo)
        if needs_cast:
            wo_sb = ldo.tile([P, 4, D], mm, tag="wo_sb")
            nc.vector.tensor_copy(wo_sb, wo_st)
        else:
            wo_sb = wo_st
        for tb in range(T // P):
            for nt in range(2):
                py = psum.tile([P, 512], F32, tag="ps_y")
                for k in range(4):
                    nc.tensor.matmul(
                        py,
                        AT[:, k, tb * P:(tb + 1) * P],
                        wo_sb[:, k, nt * 512:(nt + 1) * 512],
                        start=(k == 0),
                        stop=(k == 3),
                    )
                ysb = ypool.tile([P, 512], F32, tag="ysb")
                nc.vector.tensor_copy(ysb, py)
                nc.sync.dma_start(oa[tb, :, nt * 512:(nt + 1) * 512], ysb)

    ctx.close()


_CACHE = {}


def _get_nc(mode=None):
    mode = mode or MM_MODE
    if mode in _CACHE:
        return _CACHE[mode]
    mm = _MM_MAP[mode]
    nc = bacc.Bacc(
        "TRN2",
        target_bir_lowering=False,
        debug=False,
        enable_asserts=False,
        num_devices=N_CORES,
    )
    x_d = nc.dram_tensor("x", [T, D], F32, kind="ExternalInput")
    wqkv_d = nc.dram_tensor("w_qkv", [D, 3 * CLOC], F32, kind="ExternalInput")
    wout_d = nc.dram_tensor("w_out", [CLOC, D], F32, kind="ExternalInput")
    out_d = nc.dram_tensor("out", [T, D], F32, kind="ExternalOutput")
    with tile.TileContext(nc) as tc:
        _build_kernel_body(
            nc, tc, x_d.ap(), wqkv_d.ap(), wout_d.ap(), out_d.ap(), mm
        )
    nc.compile()
    _CACHE[mode] = nc
    return nc


def _make_in_maps(x, w_qkv, w_out):
    x = np.ascontiguousarray(np.asarray(x, dtype=np.float32))
    w_qkv = np.ascontiguousarray(np.asarray(w_qkv, dtype=np.float32))
    w_out = np.ascontiguousarray(np.asarray(w_out, dtype=np.float32))
    in_maps = []
    for c in range(N_CORES):
        b, g = divmod(c, 2)
        c0 = g * CLOC
        wloc = np.concatenate(
            [
                w_qkv[:, c0:c0 + CLOC],
                w_qkv[:, D + c0:D + c0 + CLOC],
                w_qkv[:, 2 * D + c0:2 * D + c0 + CLOC],
            ],
            axis=1,
        )
        in_maps.append({
            "x": np.ascontiguousarray(x[b]),
            "w_qkv": np.ascontiguousarray(wloc),
            "w_out": np.ascontiguousarray(w_out[c0:c0 + CLOC]),
        })
    return in_maps


def run(x, w_qkv, w_out, trace=False, mode=None):
    nc = _get_nc(mode)
    in_maps = _make_in_maps(x, w_qkv, w_out)
    res = bass_utils.run_bass_kernel_spmd(
        nc, in_maps, core_ids=list(range(N_CORES)), trace=trace
    )
    y = np.empty((B, T, D), dtype=np.float32)
    for b in range(B):
        y[b] = res.results[2 * b]["out"] + res.results[2 * b + 1]["out"]
    return y, res


def kernel(x, w_qkv, w_out):
    y, _ = run(x, w_qkv, w_out, trace=False)
    return y
